# revision 42
# baseline (speedup 1.0000x reference)
"""Trainium2 Bass kernel for nn_BidirectionalAttention (B=2, N=2048, D=2048, H=16).

Head-parallel tensor sharding across 8 NeuronCores (2 heads/core), all-bf16
matmul pipeline (fp32 PSUM accumulation):

  phase A: weight-stationary qkv projection: q^T/k^T [dh, tok] come straight
           out of PSUM (no PE transposes). Rope applied in transposed layout:
           ScalarE copies q^T PSUM->SBUF (bf16), a rotate-half matmul with a
           signed permutation matrix produces r = [-odd; even], then DVE does
           out = q*cos_dup + r*sin_dup. v computed x-stationary in natural
           [tok, dh] layout and kept SBUF-resident (no DRAM round trip).
  phase B: per (batch, head): scores s^T[k,q] = k^T.T @ q^T, exp on ScalarE
           (bf16 out), softmax denominator via DVE pair-adds (16->8 tiles)
           followed by an 8-matmul ones-chain, broadcast back to 128
           partitions with a K=1 matmul, fast reciprocal, unnormalized
           attn @ v accumulated transposed, scaled.
  phase C: output projection partial = av^T.T @ wo_rows per core (interleaved
           with phase B per batch; av stays SBUF-resident), bf16 partials.
Host: shard/transpose inputs to bf16, sum the 8 partial outputs in f32 (the
"all-reduce after wo" done at gather time).
"""

import os
import sys

sys.path.insert(0, "/opt/trn_rl_repo")

import numpy as np
import ml_dtypes

B, SEQ, DIM, NHEAD, DH = 2, 2048, 2048, 16, 128
HL = NHEAD // 8  # heads per core = 2
NCORES = 8
NT = B * SEQ  # 4096 flattened rows
SCALE = 1.0 / np.sqrt(DH)

_PROG = {}


def _build(dt_name: str):
    import concourse.tile as tile
    from concourse import bacc, mybir

    f32 = mybir.dt.float32
    dmm = {"bf16": mybir.dt.bfloat16, "f32r": mybir.dt.float32r}[dt_name]
    Exp = mybir.ActivationFunctionType.Exp

    nc = bacc.Bacc("TRN2", target_bir_lowering=False, debug=False, num_devices=NCORES)

    xt_d = nc.dram_tensor("xt", [DIM, NT], dmm, kind="ExternalInput")
    wqk_d = nc.dram_tensor("wqk", [DIM, 4 * DH], dmm, kind="ExternalInput")
    wv_d = nc.dram_tensor("wv", [DIM, HL * DH], dmm, kind="ExternalInput")
    wo_d = nc.dram_tensor("wo_r", [HL * DH, DIM], dmm, kind="ExternalInput")
    cos_d = nc.dram_tensor("cosd", [128, NT], dmm, kind="ExternalInput")
    sin_d = nc.dram_tensor("sind", [128, NT], dmm, kind="ExternalInput")
    s_d = nc.dram_tensor("srot", [128, 128], dmm, kind="ExternalInput")
    ones_d = nc.dram_tensor("ones", [128, 1], dmm, kind="ExternalInput")
    onesrow_d = nc.dram_tensor("onesrow", [1, 128], dmm, kind="ExternalInput")
    out_d = nc.dram_tensor("out_p", [NT, DIM], dmm, kind="ExternalOutput")

    with tile.TileContext(nc) as tc:
        with (
            nc.allow_low_precision(reason="bf16 matmul pipeline"),
            tc.tile_pool(name="const", bufs=1) as cp,
        ):
            s_sb = cp.tile([128, 128], dmm)
            ones = cp.tile([128, 1], dmm)
            onesrow = cp.tile([1, 128], dmm)
            # q^T / k^T SBUF-resident across phases: [tensor t][128 dh, NT]
            qkt_res = [
                cp.tile([128, NT], dmm, name=f"qktres{t}", tag=f"qktres{t}")
                for t in range(4)
            ]
            # v natural layout, SBUF-resident: [128 tok%128, 32 tile, 2*DH]
            v_all = cp.tile([128, NT // 128, HL * DH], dmm, name="vall", tag="vall")

            # ---------------- Phase A: qkv projection + rope ----------------
            GW = 512  # tokens per group
            NG = NT // GW  # 8 groups
            with (
                tc.tile_pool(name="aconst", bufs=1) as ac,
                tc.tile_pool(name="axs", bufs=2) as axs,
                tc.tile_pool(name="awork", bufs=3) as aw,
                tc.tile_pool(name="aqk", bufs=4, space="PSUM") as aqk,
                tc.tile_pool(name="arot", bufs=2, space="PSUM") as arot,
                tc.tile_pool(name="avp", bufs=2, space="PSUM") as avp,
            ):
                wqk_sb = ac.tile([128, 16, 4 * DH], dmm)
                wqk_src = wqk_d.rearrange("(c p) m -> p c m", p=128)
                wv_sb = ac.tile([128, 16, HL * DH], dmm)
                wv_src = wv_d.rearrange("(c p) m -> p c m", p=128)
                cos_sb = ac.tile([128, NT], dmm)
                sin_sb = ac.tile([128, NT], dmm)
                xt_all = xt_d.rearrange("(c p) n -> p c n", p=128)
                xs0 = axs.tile([128, 16, GW], dmm, tag="xs", bufs=3)
                for cc in range(16):
                    nc.sync.dma_start(wqk_sb[:, cc, :], wqk_src[:, cc, :])
                    nc.gpsimd.dma_start(xs0[:, cc, :], xt_all[:, cc, 0:GW])
                    nc.scalar.dma_start(wv_sb[:, cc, :], wv_src[:, cc, :])
                nc.sync.dma_start(s_sb, s_d[:, :])
                nc.sync.dma_start(ones, ones_d[:, :])
                nc.sync.dma_start(onesrow, onesrow_d[:, :])
                # rope tables arrive one group ahead of use
                nc.sync.dma_start(cos_sb[:, 0:GW], cos_d[:, 0:GW])
                nc.sync.dma_start(sin_sb[:, 0:GW], sin_d[:, 0:GW])

                xs_pend = {0: xs0}

                def load_xs(g):
                    xs = axs.tile([128, 16, GW], dmm, tag="xs", bufs=3)
                    xt_src = xt_all[:, :, g * GW : (g + 1) * GW]
                    for cg in range(4):
                        nc.gpsimd.dma_start(
                            xs[:, 4 * cg : 4 * cg + 4, :],
                            xt_src[:, 4 * cg : 4 * cg + 4, :],
                        )
                    xs_pend[g] = xs

                load_xs(1)
                for g in range(NG):
                    g0 = g * GW
                    if g + 1 < NG:
                        nl = slice((g + 1) * GW, (g + 2) * GW)
                        nc.sync.dma_start(cos_sb[:, nl], cos_d[:, nl])
                        nc.sync.dma_start(sin_sb[:, nl], sin_d[:, nl])
                    if g + 2 < NG:
                        load_xs(g + 2)
                    xs = xs_pend.pop(g)
                    # q0 q1 k0 k1 (dh on partitions), weight-stationary.
                    # All 4 accumulations first; the rotate-half matmuls run
                    # after, so the PE never waits on the ScalarE copies.
                    tmps = []
                    for t in range(4):
                        qps = aqk.tile([128, GW], f32, tag="qk", name=f"qps{t}")
                        for cc in range(16):
                            nc.tensor.matmul(
                                qps,
                                wqk_sb[:, cc, t * 128 : (t + 1) * 128],
                                xs[:, cc, :],
                                start=(cc == 0),
                                stop=(cc == 15),
                            )
                        tmp = aw.tile([128, GW], dmm, tag="tmp", bufs=5)
                        nc.scalar.copy(tmp, qps)
                        tmps.append(tmp)
                    for t in range(4):
                        tmp = tmps[t]
                        rps = arot.tile([128, GW], f32, tag="rot", name="rps")
                        nc.tensor.matmul(rps, s_sb, tmp, start=True, stop=True)
                        m1 = aw.tile([128, GW], dmm, tag="m1", bufs=2)
                        nc.vector.tensor_mul(m1, tmp, cos_sb[:, g0 : g0 + GW])
                        m2 = aw.tile([128, GW], dmm, tag="m2", bufs=2)
                        nc.vector.tensor_mul(m2, rps, sin_sb[:, g0 : g0 + GW])
                        nc.vector.tensor_add(qkt_res[t][:, g0 : g0 + GW], m1, m2)
                    # v natural layout, x-stationary
                    for st in range(GW // 128):
                        vps = avp.tile([128, HL * DH], f32, tag="v")
                        for cc in range(16):
                            nc.tensor.matmul(
                                vps,
                                xs[:, cc, st * 128 : (st + 1) * 128],
                                wv_sb[:, cc, :],
                                start=(cc == 0),
                                stop=(cc == 15),
                            )
                        nc.scalar.copy(v_all[:, g * (GW // 128) + st, :], vps)

            # ---------- Phase B+C: attention + output projection ------------
            with (
                tc.tile_pool(name="bprobs", bufs=2) as bp,
                tc.tile_pool(name="btree", bufs=2) as btr,
                tc.tile_pool(name="bwork", bufs=3) as bw,
                tc.tile_pool(name="bavres", bufs=4) as bav_sb,
                tc.tile_pool(name="cot", bufs=2) as cot,
                tc.tile_pool(name="bs", bufs=2, space="PSUM") as bs,
                tc.tile_pool(name="bmisc", bufs=1, space="PSUM") as bsm,
                tc.tile_pool(name="bav", bufs=3, space="PSUM") as bav,
            ):
                wo_sb = bp.tile([128, HL, DIM], dmm, tag="wo", bufs=1)
                nc.sync.dma_start(wo_sb, wo_d.rearrange("(j p) o -> p j o", p=128))
                avres = {}

                def emit_b_qtile(b, j, qt_i, kt_sb, qt_sb, av_r):
                    q0 = qt_i * 512
                    probs = bp.tile([128, 16, 512], dmm, tag="probs", name="probs")
                    ps8 = btr.tile([128, 8, 512], dmm, tag="ps8", name="ps8")
                    for kp in range(8):
                        sps = bs.tile([128, 2, 512], f32, tag="s", name="sps")
                        for u in range(2):
                            kt_i = 2 * kp + u
                            nc.tensor.matmul(
                                sps[:, u, :],
                                kt_sb[:, kt_i * 128 : (kt_i + 1) * 128],
                                qt_sb[:, q0 : q0 + 512],
                                start=True,
                                stop=True,
                            )
                        nc.scalar.activation(probs[:, 2 * kp : 2 * kp + 2, :], sps, Exp)
                        # incremental level-1 pair-add keeps the post-exp tail short
                        nc.vector.tensor_add(
                            ps8[:, kp, :], probs[:, 2 * kp, :], probs[:, 2 * kp + 1, :]
                        )
                    # av matmuls first: they only need probs, so the PE keeps
                    # streaming while the DVE finishes the k-tile reduction
                    avps = bav.tile([128, 512], f32, tag="av", name="avps")
                    for cc in range(16):
                        nc.tensor.matmul(
                            avps,
                            v_all[:, b * 16 + cc, j * DH : (j + 1) * DH],
                            probs[:, cc, :],
                            start=(cc == 0),
                            stop=(cc == 15),
                        )
                    # finish k-tile reduction on DVE (slice-halving), then a
                    # single ones-matmul for the partition sum + broadcast
                    tb = btr.tile([128, 4, 512], dmm, tag="tb", name="tb")
                    nc.vector.tensor_add(tb, ps8[:, 0:4, :], ps8[:, 4:8, :])
                    tc2 = btr.tile([128, 2, 512], dmm, tag="tc", name="tc2")
                    nc.vector.tensor_add(tc2, tb[:, 0:2, :], tb[:, 2:4, :])
                    sumb = btr.tile([128, 512], dmm, tag="sumb", name="sumb")
                    nc.vector.tensor_add(sumb, tc2[:, 0, :], tc2[:, 1, :])
                    sum_ps = bsm.tile([1, 512], f32, tag="sum", name="sum_ps")
                    nc.tensor.matmul(sum_ps, ones, sumb, start=True, stop=True)
                    sum_sb = bw.tile([1, 512], dmm, tag="sumsb", name="sum_sb")
                    nc.vector.tensor_copy(sum_sb, sum_ps)
                    rbc_ps = bsm.tile([128, 512], f32, tag="sum", name="rbc_ps")
                    nc.tensor.matmul(rbc_ps, onesrow, sum_sb, start=True, stop=True)
                    rbc = bw.tile([128, 512], f32, tag="rbcsb", name="rbc")
                    nc.vector.reciprocal_approx_fast(rbc, rbc_ps)
                    nc.vector.tensor_mul(av_r[:, q0 : q0 + 512], avps, rbc)

                def emit_c_subtile(b, nl, tail=False):
                    g0 = b * SEQ + nl * 128
                    ot = cot.tile([128, DIM], dmm, tag="ot", name="ot")
                    for do in range(4):
                        ops = bav.tile([128, 512], f32, tag="av", name="ops")
                        for j in range(HL):
                            nc.tensor.matmul(
                                ops,
                                avres[(b, j)][:, nl * 128 : (nl + 1) * 128],
                                wo_sb[:, j, do * 512 : (do + 1) * 512],
                                start=(j == 0),
                                stop=(j == 1),
                            )
                        osl = ot[:, do * 512 : (do + 1) * 512]
                        if do % 2 == 0 if tail else do < 2:
                            nc.scalar.copy(osl, ops)
                        else:
                            nc.vector.tensor_copy(osl, ops)
                        if tail:
                            nc.sync.dma_start(
                                out_d[g0 : g0 + 128, do * 512 : (do + 1) * 512], osl
                            )
                    if not tail:
                        nc.sync.dma_start(out_d[g0 : g0 + 128, :], ot)

                # attention per (batch, head); C(b) interleaved into B(b, j=1):
                # C(b, 4*qt..4*qt+3) reads exactly the av columns written by
                # qtile qt of both heads, so it can follow immediately.
                for b in range(B):
                    for j in range(HL):
                        kt_sb = qkt_res[2 + j][:, b * SEQ : (b + 1) * SEQ]
                        qt_sb = qkt_res[j][:, b * SEQ : (b + 1) * SEQ]
                        av_r = bav_sb.tile([128, SEQ], dmm, tag="avres", name="av_r")
                        avres[(b, j)] = av_r
                        for qt_i in range(4):
                            emit_b_qtile(b, j, qt_i, kt_sb, qt_sb, av_r)
                            if j == 1:
                                for u in range(4):
                                    tail = b == 1 and qt_i == 3 and u >= 2
                                    emit_c_subtile(b, 4 * qt_i + u, tail=tail)

    nc.compile()
    return nc


def _get_prog():
    dt_name = os.environ.get("KMM_DT", "bf16")
    key = ("prog", dt_name)
    if key not in _PROG:
        _PROG[key] = _build(dt_name)
    return _PROG[key], dt_name


def _shard(x, freqs_cis, wqkv, wo, dt_name):
    np_dt = {"bf16": ml_dtypes.bfloat16, "f32r": np.float32}[dt_name]

    def rnd(a):
        return np.ascontiguousarray(a, dtype=np.float32).astype(np_dt)

    x = np.asarray(x, dtype=np.float32)
    freqs_cis = np.asarray(freqs_cis, dtype=np.float32)
    wqkv = np.asarray(wqkv, dtype=np.float32)
    wo = np.asarray(wo, dtype=np.float32)

    xt = rnd(x.reshape(NT, DIM).T)

    # rope tables in transposed layout: [128 partitions (dh after perm), NT]
    # partition p<64 -> even component, freq p; p>=64 -> odd component, freq p-64
    cos = freqs_cis[:, :, 0].T  # [64, SEQ]
    sin = freqs_cis[:, :, 1].T
    cosb = np.concatenate([cos] * B, axis=1)  # [64, NT]
    sinb = np.concatenate([sin] * B, axis=1)
    cosd = rnd(np.concatenate([cosb, cosb], axis=0))  # [128, NT]
    sind = rnd(np.concatenate([sinb, sinb], axis=0))

    # rotate-half permutation (as lhsT): r = [-odd; even]
    s_rot = np.zeros((128, 128), np.float32)
    s_rot[np.arange(64), np.arange(64) + 64] = 1.0  # r[m>=64] = +even
    s_rot[np.arange(64) + 64, np.arange(64)] = -1.0  # r[m<64] = -odd
    s_rot = rnd(s_rot)

    perm = np.concatenate([np.arange(0, DH, 2), np.arange(1, DH, 2)])  # de-interleave
    consts = {
        "srot": s_rot,
        "ones": rnd(np.ones((128, 1), np.float32)),
        "onesrow": rnd(np.ones((1, 128), np.float32)),
    }
    in_maps = []
    for c in range(NCORES):
        h0 = c * HL
        wq = [wqkv[:, h * DH : (h + 1) * DH][:, perm] * SCALE for h in (h0, h0 + 1)]
        wk = [wqkv[:, DIM + h * DH : DIM + (h + 1) * DH][:, perm] for h in (h0, h0 + 1)]
        wqk_c = rnd(np.concatenate(wq + wk, axis=1))  # [DIM, 512]
        wv_c = rnd(wqkv[:, 2 * DIM + h0 * DH : 2 * DIM + (h0 + HL) * DH])  # [DIM, 256]
        wo_c = rnd(wo[h0 * DH : (h0 + HL) * DH, :])  # [256, DIM]
        in_maps.append(
            {
                "xt": xt,
                "wqk": wqk_c,
                "wv": wv_c,
                "wo_r": wo_c,
                "cosd": cosd,
                "sind": sind,
                **consts,
            }
        )
    return in_maps


def _run(in_maps, trace=False, **kw):
    from concourse.bass_utils import run_bass_kernel_spmd

    prog, _ = _get_prog()
    return run_bass_kernel_spmd(prog, in_maps, list(range(NCORES)), trace=trace, **kw)


def kernel(x, freqs_cis, wqkv, wo):
    _, dt_name = _get_prog()
    in_maps = _shard(x, freqs_cis, wqkv, wo, dt_name)
    res = _run(in_maps, trace=False)
    acc = np.zeros((NT, DIM), dtype=np.float32)
    for c in range(NCORES):
        acc += np.asarray(res.results[c]["out_p"]).astype(np.float32)
    return acc.reshape(B, SEQ, DIM)
